# revision 29
# baseline (speedup 1.0000x reference)
"""ColorCNN (vq_codebook) Trainium2 kernel, v2.

Math (per image b):
    h      = relu(w1^T feat + b1)             (512, H*W)
    logits = w2^T h + b2                      (64, H*W)
    mask   = softmax_k(logits)
    wc[c,k]= sum_p img[c,p] mask[k,p] / HW
    out    = sum_k mask[k,p] wc[c,k]
    stats: mean over (b,k) of max_p mask; mean over b of std_k(mean_p mask)

Sharding: pure data parallel, one image per NeuronCore. Weights replicated.
Cross-image stat reductions finish exactly on the host.

Per-core dataflow (channel-major, pixel tiles of 512, processed in pairs):
  GEMM1 f32r, row-packed 2x on the PE (K=64 in row groups 0/64, feat
  replicated to both partition halves by an idle-GPSIMD f32r-rounding copy)
  relu split ACT/DVE -> h f32r; GEMM2 f32r (4 accumulating matmuls);
  ACT exp(+b2) -> exp f32r; 8 PE transposes -> (pixel, k) psum;
  one DVE 3D reduce + reciprocal -> per-pixel 1/sum; one DVE broadcast
  multiply -> bf16 mask (p,k); per-k max via GPSIMD partition_all_reduce
  (DMA'd out per pair, host-reduced); wc accumulation matmuls with
  [img^T | 1] (ones column yields per-k sums for free); 4 paired PE
  transposes back -> (k,p) bf16 mask stored in SBUF (128, 32768) with the
  pair's two tiles on partition halves 0-63/64-127 (no DRAM round trip).
Pass 2: per pair two matmuls wc^T @ mask (bases 0 and 64) -> (3, 512)
  psum -> DVE/ACT copies -> batched DMA out.
"""
import sys

if "/opt/trn_rl_repo" not in sys.path:
    sys.path.insert(0, "/opt/trn_rl_repo")

import numpy as np

import concourse.bacc as bacc
import concourse.mybir as mybir
import concourse.tile as tile
from concourse import bass_isa
from concourse.bass_utils import run_bass_kernel_spmd
from concourse.masks import make_identity

F32 = mybir.dt.float32
F32R = mybir.dt.float32r
BF16 = mybir.dt.bfloat16
AF = mybir.ActivationFunctionType
ALU = mybir.AluOpType

B, CF, H, W = 8, 64, 256, 256
HID, K = 512, 64
P = H * W              # 65536 pixels per image
NP = 512               # pixels per tile
NT = P // NP           # 128 tiles
NPAIR = NT // 2        # 64 tile pairs
BS = 4                 # tiles per feat/img/out DMA batch
RELU_ACT2 = 896        # extra relu elems on ACT beyond the first half

_CACHED = {}


def _build(use_bias):
    nc = bacc.Bacc("TRN2", target_bir_lowering=False, debug=False, num_devices=8)

    feat_d = nc.dram_tensor("feat", [CF, P], F32, kind="ExternalInput").ap()
    img_d = nc.dram_tensor("img", [3, P], F32, kind="ExternalInput").ap()
    w1a_d = nc.dram_tensor("w1a", [CF + 1, HID], F32, kind="ExternalInput").ap()
    w2_d = nc.dram_tensor("w2", [HID, K], F32, kind="ExternalInput").ap()
    b2_d = nc.dram_tensor("b2", [K, 1], F32, kind="ExternalInput").ap()
    ones_d = nc.dram_tensor("onesrow", [1, BS * NP], F32, kind="ExternalInput").ap()

    out_d = nc.dram_tensor("out", [3, P], F32, kind="ExternalOutput").ap()
    pmax_d = nc.dram_tensor("pmax", [NPAIR, 8 * K], BF16, kind="ExternalOutput").ap()
    sums_d = nc.dram_tensor("sums", [1, K], F32, kind="ExternalOutput").ap()

    KG = CF + 1 if use_bias else CF  # GEMM1 contraction size

    with tile.TileContext(nc) as tc:
        with (
            tc.tile_pool(name="const", bufs=1) as cpool,
            tc.tile_pool(name="store", bufs=1) as spool,
            tc.tile_pool(name="feat", bufs=2) as fpool,
            tc.tile_pool(name="featr", bufs=2) as frpool,
            tc.tile_pool(name="hsb", bufs=3) as hpool,
            tc.tile_pool(name="exps", bufs=4) as epool,
            tc.tile_pool(name="exp2", bufs=2) as e2pool,
            tc.tile_pool(name="smalls", bufs=3) as smpool,
            tc.tile_pool(name="imgs", bufs=2) as ipool,
            tc.tile_pool(name="outs", bufs=3) as opool,
            tc.tile_pool(name="psh", bufs=2, space="PSUM") as psh,
            tc.tile_pool(name="psmisc", bufs=3, space="PSUM") as psm,
            tc.tile_pool(name="pswc", bufs=1, space="PSUM") as pswc,
        ):
            # ---- constants / weights ----
            w1s_f = cpool.tile([KG, HID], F32)
            nc.sync.dma_start(w1s_f[:], w1a_d[0:KG, :])
            w1s = cpool.tile([KG, HID], F32R)
            nc.vector.tensor_copy(w1s[:], w1s_f[:])

            w2_f = cpool.tile([128, 4, K], F32)
            nc.sync.dma_start(w2_f[:], w2_d.rearrange("(j d) k -> d j k", j=4))
            w2r = cpool.tile([128, 4, K], F32R)
            nc.vector.tensor_copy(w2r[:], w2_f[:])

            b2_t = cpool.tile([K, 1], F32)
            nc.sync.dma_start(b2_t[:], b2_d[:])

            id64f = cpool.tile([K, K], F32)
            make_identity(nc, id64f[:])
            id64r = cpool.tile([K, K], F32R)
            nc.vector.tensor_copy(id64r[:], id64f[:])
            id3 = cpool.tile([3, 3], F32)
            make_identity(nc, id3[:])
            id128b = cpool.tile([128, 128], BF16)
            make_identity(nc, id128b[:])
            id4b = cpool.tile([4, 4], BF16)
            make_identity(nc, id4b[:])

            # mask store: pair p at columns p*512, tile A on partitions 0-63,
            # tile B on partitions 64-127; bf16
            mask_store = spool.tile([128, NPAIR * NP], BF16)

            wc_ps = pswc.tile([4, K], F32)

            # ---- pass 1 (pair loop) ----
            for p in range(NPAIR):
                bi = (2 * p) % BS  # tile index within DMA batch
                if bi == 0:
                    c0b = p * 2 * NP
                    feat_s = fpool.tile([KG, BS * NP], F32)
                    nc.sync.dma_start(
                        feat_s[0:CF, :], feat_d[:, c0b:c0b + BS * NP]
                    )
                    if use_bias:
                        nc.sync.dma_start(feat_s[CF:CF + 1, :], ones_d[:])
                    feat_r = frpool.tile([KG, BS * NP], F32R)
                    nc.gpsimd.tensor_copy(feat_r[:], feat_s[:])

                    img_t = ipool.tile([3, BS * NP], F32)
                    nc.gpsimd.dma_start(img_t[:], img_d[:, c0b:c0b + BS * NP])

                lg_pair = []
                for ti in range(2):
                    t = 2 * p + ti
                    foff = (bi + ti) * NP

                    # GEMM1 in two halves (d-chunks 0,1 then 2,3), each into
                    # its own 2-bank psum tile; relu = one ACT + one DVE op
                    h_sb = hpool.tile([128, 4 * NP], F32R)
                    for half in range(2):
                        h_ps = psh.tile([128, 2 * NP], F32, tag="hps")
                        for dj2 in range(2):
                            dj = 2 * half + dj2
                            nc.tensor.matmul(
                                h_ps[:, dj2 * NP:(dj2 + 1) * NP],
                                w1s[:, dj * 128:(dj + 1) * 128],
                                feat_r[:, foff:foff + NP],
                                start=True, stop=True,
                            )
                        if half == 0:
                            nc.scalar.activation(
                                h_sb[:, 0:2 * NP], h_ps[:], AF.Relu
                            )
                        else:
                            nc.scalar.activation(
                                h_sb[:, 2 * NP:2 * NP + RELU_ACT2],
                                h_ps[:, 0:RELU_ACT2], AF.Relu,
                            )
                            nc.vector.tensor_scalar_max(
                                h_sb[:, 2 * NP + RELU_ACT2:4 * NP],
                                h_ps[:, RELU_ACT2:2 * NP], 0.0,
                            )

                    # GEMM2
                    log_ps = psm.tile([K, NP], F32, tag="misc")
                    for dj in range(4):
                        nc.tensor.matmul(
                            log_ps[:],
                            w2r[:, dj, :],
                            h_sb[:, dj * NP:(dj + 1) * NP],
                            start=(dj == 0), stop=(dj == 3),
                        )
                    lg_sb = epool.tile([K, NP], F32R)
                    nc.scalar.activation(
                        lg_sb[:], log_ps[:], AF.Exp, bias=b2_t[:], scale=1.0
                    )
                    lg_pair.append(lg_sb)

                # transpose both tiles into (pixel, jj*64+k) chunks
                # chunk order within the pair is (j, tile, k) so that the
                # back-transposes can take contiguous (128, 128) slices
                expT_ps = psm.tile([128, 8 * K], F32R, tag="misc")
                for jj in range(8):
                    j, ti = jj // 2, jj % 2
                    nc.tensor.transpose(
                        expT_ps[:, jj * K:(jj + 1) * K],
                        lg_pair[ti][:, j * 128:(j + 1) * 128],
                        id64r[:],
                    )
                exp_fp = expT_ps[:].bitcast(F32)
                # softmax denominators + normalize (one op each)
                scol = smpool.tile([128, 8], F32)
                nc.vector.reduce_sum(
                    scol[:],
                    exp_fp.rearrange("p (j k) -> p j k", j=8),
                    axis=mybir.AxisListType.X,
                )
                rcol = smpool.tile([128, 8], F32)
                nc.vector.reciprocal(rcol[:], scol[:])
                mask_pk = smpool.tile([128, 8 * K], BF16)
                nc.vector.tensor_tensor(
                    out=mask_pk[:].rearrange("p (j k) -> p j k", j=8),
                    in0=exp_fp.rearrange("p (j k) -> p j k", j=8),
                    in1=rcol[:].unsqueeze(2).broadcast_to([128, 8, K]),
                    op=ALU.mult,
                )

                # per-k max stat: partition reduce on gpsimd, row DMA'd out
                pm = smpool.tile([128, 8 * K], BF16)
                nc.gpsimd.partition_all_reduce(
                    pm[:], mask_pk[:], channels=128,
                    reduce_op=bass_isa.ReduceOp.max,
                )
                nc.gpsimd.dma_start(pmax_d[p:p + 1, :], pm[0:1, :])

                # img^T chunks + ones column
                ig_ps = psm.tile([128, 8 * 3], F32, tag="misc")
                for jj in range(8):
                    poff = (bi + jj % 2) * NP + (jj // 2) * 128
                    nc.tensor.transpose(
                        ig_ps[:, jj * 3:(jj + 1) * 3],
                        img_t[:, poff:poff + 128],
                        id3[:],
                    )
                imgaug = ipool.tile([128, 8, 4], BF16)
                nc.vector.memset(imgaug[:, :, 3:4], 1.0)
                nc.vector.tensor_copy(
                    imgaug[:, :, 0:3],
                    ig_ps[:].rearrange("p (j c) -> p j c", j=8),
                )

                # wc accumulation (row 3 = per-k sums via the ones column)
                for jj in range(8):
                    nc.tensor.matmul(
                        wc_ps[:],
                        imgaug[:, jj, :],
                        mask_pk[:, jj * K:(jj + 1) * K],
                        start=(p == 0 and jj == 0),
                        stop=(p == NPAIR - 1 and jj == 7),
                    )

                # transpose back to (k, p): paired chunks (jA, jA+4) so tile A
                # lands on psum partitions 0-63 and tile B on 64-127
                mkp_ps = psm.tile([128, NP], BF16, tag="misc")
                for j in range(4):
                    nc.tensor.transpose(
                        mkp_ps[:, j * 128:(j + 1) * 128],
                        mask_pk[:, (2 * j) * K:(2 * j + 2) * K],
                        id128b[:],
                    )
                nc.vector.tensor_copy(
                    mask_store[:, p * NP:(p + 1) * NP], mkp_ps[:]
                )

            # ---- finalize wc + stats ----
            wc_sb = cpool.tile([4, K], F32)
            nc.vector.tensor_copy(wc_sb[:], wc_ps[:])
            nc.sync.dma_start(sums_d[:], wc_sb[3:4, :])
            # wc, scaled by 1/HW, duplicated into both column halves so the
            # transpose lands copies on psum partitions 0-63 and 64-127
            wc_bf = cpool.tile([4, 128], BF16)
            nc.scalar.mul(wc_bf[:, 0:K], wc_sb[:], 1.0 / P)
            nc.scalar.mul(wc_bf[:, K:128], wc_sb[:], 1.0 / P)
            wcT_ps = psm.tile([128, 4], BF16, tag="misc")
            nc.tensor.transpose(wcT_ps[:], wc_bf[:], id4b[:])
            wcT_sb = cpool.tile([128, 4], BF16)
            nc.vector.tensor_copy(wcT_sb[:], wcT_ps[:])

            # ---- pass 2 ----
            for p in range(NPAIR):
                bi = (2 * p) % BS
                if bi == 0:
                    out_sb2 = opool.tile([3, BS * NP], F32, tag="out_sb")
                out_ps = psh.tile([3, 2 * NP], F32, tag="hps")
                for ti in range(2):
                    lo = ti * CF
                    nc.tensor.matmul(
                        out_ps[:, ti * NP:(ti + 1) * NP],
                        wcT_sb[lo:lo + K, 0:3],
                        mask_store[lo:lo + K, p * NP:(p + 1) * NP],
                        start=True, stop=True,
                    )
                doff = bi * NP
                if p % 2 == 0:
                    nc.vector.tensor_copy(
                        out_sb2[:, doff:doff + 2 * NP], out_ps[:]
                    )
                else:
                    nc.scalar.copy(out_sb2[:, doff:doff + 2 * NP], out_ps[:])
                if bi + 2 == BS:
                    c0b = (2 * p - BS + 2) * NP
                    eng = [nc.sync, nc.gpsimd][(p // 2) % 2]
                    eng.dma_start(out_d[:, c0b:c0b + BS * NP], out_sb2[:])

    nc.compile()
    return nc


def _get_nc(use_bias):
    if use_bias not in _CACHED:
        _CACHED[use_bias] = _build(use_bias)
    return _CACHED[use_bias]


def kernel(img, feat, coord_map, w1, b1, w2, b2, _want_trace=False):
    img = np.ascontiguousarray(np.asarray(img, dtype=np.float32))
    feat = np.ascontiguousarray(np.asarray(feat, dtype=np.float32))
    w1 = np.asarray(w1, dtype=np.float32)
    b1 = np.asarray(b1, dtype=np.float32)
    w2 = np.ascontiguousarray(np.asarray(w2, dtype=np.float32))
    b2 = np.asarray(b2, dtype=np.float32)

    use_bias = bool(np.any(b1 != 0.0))
    w1a = np.ascontiguousarray(np.concatenate([w1, b1[None, :]], axis=0))
    onesrow = np.ones((1, BS * NP), dtype=np.float32)
    b2c = np.ascontiguousarray(b2[:, None])

    nc = _get_nc(use_bias)
    in_maps = [
        {
            "feat": feat[i].reshape(CF, P),
            "img": img[i].reshape(3, P),
            "w1a": w1a,
            "w2": w2,
            "b2": b2c,
            "onesrow": onesrow,
        }
        for i in range(B)
    ]
    res = run_bass_kernel_spmd(nc, in_maps, list(range(B)), trace=_want_trace)

    transformed = np.empty((B, 3, H, W), dtype=np.float32)
    maxes = np.empty((B, K), dtype=np.float32)
    stds = np.empty((B,), dtype=np.float64)
    for i in range(B):
        r = res.results[i]
        transformed[i] = r["out"].reshape(3, H, W)
        pm = np.asarray(r["pmax"]).astype(np.float32)  # (NPAIR, 8*K) from bf16
        maxes[i] = pm.reshape(NPAIR * 8, K).max(axis=0)
        mean_k = r["sums"].reshape(K).astype(np.float64) / P
        stds[i] = np.std(mean_k, ddof=1)

    mean_max = np.float32(maxes.astype(np.float64).mean())
    std_mean = np.float32(stds.mean())
    out = (transformed, mean_max, std_mean)
    if _want_trace:
        return out, res
    return out


# revision 38
# speedup vs baseline: 1.0123x; 1.0123x over previous
"""ColorCNN (vq_codebook) Trainium2 kernel, v2.

Math (per image b):
    h      = relu(w1^T feat + b1)             (512, H*W)
    logits = w2^T h + b2                      (64, H*W)
    mask   = softmax_k(logits)
    wc[c,k]= sum_p img[c,p] mask[k,p] / HW
    out    = sum_k mask[k,p] wc[c,k]
    stats: mean over (b,k) of max_p mask; mean over b of std_k(mean_p mask)

Sharding: pure data parallel, one image per NeuronCore. Weights replicated.
Cross-image stat reductions finish exactly on the host.

Per-core dataflow (channel-major, pixel tiles of 512, processed in pairs):
  GEMM1 f32r, row-packed 2x on the PE (K=64 in row groups 0/64, feat
  replicated to both partition halves by an idle-GPSIMD f32r-rounding copy)
  relu split ACT/DVE -> h f32r; GEMM2 f32r (4 accumulating matmuls);
  ACT exp(+b2) -> exp f32r; 8 PE transposes -> (pixel, k) psum;
  one DVE 3D reduce + reciprocal -> per-pixel 1/sum; one DVE broadcast
  multiply -> bf16 mask (p,k); per-k max via GPSIMD partition_all_reduce
  (DMA'd out per pair, host-reduced); wc accumulation matmuls with
  [img^T | 1] (ones column yields per-k sums for free); 4 paired PE
  transposes back -> (k,p) bf16 mask stored in SBUF (128, 32768) with the
  pair's two tiles on partition halves 0-63/64-127 (no DRAM round trip).
Pass 2: per pair two matmuls wc^T @ mask (bases 0 and 64) -> (3, 512)
  psum -> DVE/ACT copies -> batched DMA out.
"""
import sys

if "/opt/trn_rl_repo" not in sys.path:
    sys.path.insert(0, "/opt/trn_rl_repo")

import numpy as np

import concourse.bacc as bacc
import concourse.mybir as mybir
import concourse.tile as tile
from concourse import bass_isa
from concourse.bass_utils import run_bass_kernel_spmd
from concourse.masks import make_identity

F32 = mybir.dt.float32
F32R = mybir.dt.float32r
BF16 = mybir.dt.bfloat16
AF = mybir.ActivationFunctionType
ALU = mybir.AluOpType

B, CF, H, W = 8, 64, 256, 256
HID, K = 512, 64
P = H * W              # 65536 pixels per image
NP = 512               # pixels per tile
NT = P // NP           # 128 tiles
NPAIR = NT // 2        # 64 tile pairs
BS = 4                 # tiles per feat/img DMA batch
OBS = 4                # tiles per out DMA batch
RELU_ACT2 = 896        # extra relu elems on ACT beyond the first half

_CACHED = {}


def _build(use_bias):
    nc = bacc.Bacc("TRN2", target_bir_lowering=False, debug=False, num_devices=8)

    feat_d = nc.dram_tensor("feat", [CF, P], F32, kind="ExternalInput").ap()
    img_d = nc.dram_tensor("img", [3, P], F32, kind="ExternalInput").ap()
    w1a_d = nc.dram_tensor("w1a", [CF + 1, HID], F32, kind="ExternalInput").ap()
    w2_d = nc.dram_tensor("w2", [HID, K], F32, kind="ExternalInput").ap()
    b2_d = nc.dram_tensor("b2", [K, 1], F32, kind="ExternalInput").ap()
    ones_d = nc.dram_tensor("onesrow", [1, BS * NP], F32, kind="ExternalInput").ap()

    out_d = nc.dram_tensor("out", [3, P], F32, kind="ExternalOutput").ap()
    pmax_d = nc.dram_tensor("pmax", [NPAIR, 8 * K], BF16, kind="ExternalOutput").ap()
    sums_d = nc.dram_tensor("sums", [1, K], F32, kind="ExternalOutput").ap()

    KG = CF + 1 if use_bias else CF  # GEMM1 contraction size

    with tile.TileContext(nc) as tc:
        with (
            tc.tile_pool(name="const", bufs=1) as cpool,
            tc.tile_pool(name="store", bufs=1) as spool,
            tc.tile_pool(name="feat", bufs=2) as fpool,
            tc.tile_pool(name="featr", bufs=2) as frpool,
            tc.tile_pool(name="hsb", bufs=3) as hpool,
            tc.tile_pool(name="exps", bufs=4) as epool,
            tc.tile_pool(name="exp2", bufs=2) as e2pool,
            tc.tile_pool(name="smalls", bufs=3) as smpool,
            tc.tile_pool(name="imgs", bufs=2) as ipool,
            tc.tile_pool(name="outs", bufs=4) as opool,
            tc.tile_pool(name="psh", bufs=2, space="PSUM") as psh,
            tc.tile_pool(name="psmisc", bufs=3, space="PSUM") as psm,
            tc.tile_pool(name="pswc", bufs=1, space="PSUM") as pswc,
        ):
            # ---- constants / weights ----
            w1s_f = cpool.tile([KG, HID], F32)
            nc.sync.dma_start(w1s_f[:], w1a_d[0:KG, :])
            w1s = cpool.tile([KG, HID], F32R)
            nc.vector.tensor_copy(w1s[:], w1s_f[:])

            w2_f = cpool.tile([128, 4, K], F32)
            nc.sync.dma_start(w2_f[:], w2_d.rearrange("(j d) k -> d j k", j=4))
            w2r = cpool.tile([128, 4, K], F32R)
            nc.vector.tensor_copy(w2r[:], w2_f[:])

            b2_t = cpool.tile([K, 1], F32)
            nc.sync.dma_start(b2_t[:], b2_d[:])

            id64f = cpool.tile([K, K], F32)
            make_identity(nc, id64f[:])
            id64r = cpool.tile([K, K], F32R)
            nc.vector.tensor_copy(id64r[:], id64f[:])
            id3 = cpool.tile([3, 3], F32)
            make_identity(nc, id3[:])
            id128b = cpool.tile([128, 128], BF16)
            make_identity(nc, id128b[:])
            id4b = cpool.tile([4, 4], BF16)
            make_identity(nc, id4b[:])

            # mask store: pair p at columns p*512, tile A on partitions 0-63,
            # tile B on partitions 64-127; bf16
            mask_store = spool.tile([128, NPAIR * NP], BF16)

            wc_ps = pswc.tile([4, K], F32)

            # ---- pass 1 (pair loop) ----
            for p in range(NPAIR):
                bi = (2 * p) % BS  # tile index within DMA batch
                if bi == 0:
                    c0b = p * 2 * NP
                    feat_s = fpool.tile([KG, BS * NP], F32)
                    nc.sync.dma_start(
                        feat_s[0:CF, :], feat_d[:, c0b:c0b + BS * NP]
                    )
                    if use_bias:
                        nc.sync.dma_start(feat_s[CF:CF + 1, :], ones_d[:])
                    feat_r = frpool.tile([KG, BS * NP], F32R)
                    nc.gpsimd.tensor_copy(feat_r[:], feat_s[:])

                    img_t = ipool.tile([3, BS * NP], F32)
                    nc.gpsimd.dma_start(img_t[:], img_d[:, c0b:c0b + BS * NP])

                lg_pair = []
                for ti in range(2):
                    t = 2 * p + ti
                    foff = (bi + ti) * NP

                    # GEMM1 in two halves (d-chunks 0,1 then 2,3), each into
                    # its own 2-bank psum tile; relu = one ACT + one DVE op
                    h_sb = hpool.tile([128, 4 * NP], F32R)
                    for half in range(2):
                        h_ps = psh.tile([128, 2 * NP], F32, tag="hps")
                        for dj2 in range(2):
                            dj = 2 * half + dj2
                            nc.tensor.matmul(
                                h_ps[:, dj2 * NP:(dj2 + 1) * NP],
                                w1s[:, dj * 128:(dj + 1) * 128],
                                feat_r[:, foff:foff + NP],
                                start=True, stop=True,
                            )
                        if half == 0:
                            nc.scalar.activation(
                                h_sb[:, 0:2 * NP], h_ps[:], AF.Relu
                            )
                        else:
                            nc.scalar.activation(
                                h_sb[:, 2 * NP:2 * NP + RELU_ACT2],
                                h_ps[:, 0:RELU_ACT2], AF.Relu,
                            )
                            nc.vector.tensor_scalar_max(
                                h_sb[:, 2 * NP + RELU_ACT2:4 * NP],
                                h_ps[:, RELU_ACT2:2 * NP], 0.0,
                            )

                    # GEMM2
                    log_ps = psm.tile([K, NP], F32, tag="misc")
                    for dj in range(4):
                        nc.tensor.matmul(
                            log_ps[:],
                            w2r[:, dj, :],
                            h_sb[:, dj * NP:(dj + 1) * NP],
                            start=(dj == 0), stop=(dj == 3),
                        )
                    lg_sb = epool.tile([K, NP], F32R)
                    nc.scalar.activation(
                        lg_sb[:], log_ps[:], AF.Exp, bias=b2_t[:], scale=1.0
                    )
                    lg_pair.append(lg_sb)

                # transpose both tiles into (pixel, jj*64+k) chunks
                # chunk order within the pair is (j, tile, k) so that the
                # back-transposes can take contiguous (128, 128) slices
                expT_ps = psm.tile([128, 8 * K], F32R, tag="misc")
                for jj in range(8):
                    j, ti = jj // 2, jj % 2
                    nc.tensor.transpose(
                        expT_ps[:, jj * K:(jj + 1) * K],
                        lg_pair[ti][:, j * 128:(j + 1) * 128],
                        id64r[:],
                    )
                exp_fp = expT_ps[:].bitcast(F32)
                # softmax denominators + normalize (one op each)
                scol = smpool.tile([128, 8], F32)
                nc.vector.reduce_sum(
                    scol[:],
                    exp_fp.rearrange("p (j k) -> p j k", j=8),
                    axis=mybir.AxisListType.X,
                )
                rcol = smpool.tile([128, 8], F32)
                nc.vector.reciprocal(rcol[:], scol[:])
                mask_pk = smpool.tile([128, 8 * K], BF16)
                nc.vector.tensor_tensor(
                    out=mask_pk[:].rearrange("p (j k) -> p j k", j=8),
                    in0=exp_fp.rearrange("p (j k) -> p j k", j=8),
                    in1=rcol[:].unsqueeze(2).broadcast_to([128, 8, K]),
                    op=ALU.mult,
                )

                # per-k max stat: partition reduce on gpsimd, row DMA'd out
                pm = smpool.tile([128, 8 * K], BF16)
                nc.gpsimd.partition_all_reduce(
                    pm[:], mask_pk[:], channels=128,
                    reduce_op=bass_isa.ReduceOp.max,
                )
                nc.gpsimd.dma_start(pmax_d[p:p + 1, :], pm[0:1, :])

                # img^T chunks + ones column
                ig_ps = psm.tile([128, 8 * 3], F32, tag="misc")
                for jj in range(8):
                    poff = (bi + jj % 2) * NP + (jj // 2) * 128
                    nc.tensor.transpose(
                        ig_ps[:, jj * 3:(jj + 1) * 3],
                        img_t[:, poff:poff + 128],
                        id3[:],
                    )
                imgaug = ipool.tile([128, 8, 4], BF16)
                nc.vector.memset(imgaug[:, :, 3:4], 1.0)
                nc.vector.tensor_copy(
                    imgaug[:, :, 0:3],
                    ig_ps[:].rearrange("p (j c) -> p j c", j=8),
                )

                # wc accumulation (row 3 = per-k sums via the ones column)
                for jj in range(8):
                    nc.tensor.matmul(
                        wc_ps[:],
                        imgaug[:, jj, :],
                        mask_pk[:, jj * K:(jj + 1) * K],
                        start=(p == 0 and jj == 0),
                        stop=(p == NPAIR - 1 and jj == 7),
                    )

                # transpose back to (k, p): paired chunks (jA, jA+4) so tile A
                # lands on psum partitions 0-63 and tile B on 64-127
                mkp_ps = psm.tile([128, NP], BF16, tag="misc")
                for j in range(4):
                    nc.tensor.transpose(
                        mkp_ps[:, j * 128:(j + 1) * 128],
                        mask_pk[:, (2 * j) * K:(2 * j + 2) * K],
                        id128b[:],
                    )
                nc.vector.tensor_copy(
                    mask_store[:, p * NP:(p + 1) * NP], mkp_ps[:]
                )

            # ---- finalize wc + stats ----
            wc_sb = cpool.tile([4, K], F32)
            nc.vector.tensor_copy(wc_sb[:], wc_ps[:])
            nc.sync.dma_start(sums_d[:], wc_sb[3:4, :])
            # wc, scaled by 1/HW, duplicated into both column halves so the
            # transpose lands copies on psum partitions 0-63 and 64-127
            wc_bf = cpool.tile([4, 128], BF16)
            nc.scalar.mul(wc_bf[:, 0:K], wc_sb[:], 1.0 / P)
            nc.scalar.mul(wc_bf[:, K:128], wc_sb[:], 1.0 / P)
            wcT_ps = psm.tile([128, 4], BF16, tag="misc")
            nc.tensor.transpose(wcT_ps[:], wc_bf[:], id4b[:])
            wcT_sb = cpool.tile([128, 4], BF16)
            nc.vector.tensor_copy(wcT_sb[:], wcT_ps[:])

            # ---- pass 2 ----
            for p in range(NPAIR):
                bi = (2 * p) % OBS
                if bi == 0:
                    out_sb2 = opool.tile([3, OBS * NP], F32, tag="out_sb")
                out_ps = psh.tile([3, 2 * NP], F32, tag="hps")
                for ti in range(2):
                    lo = ti * CF
                    nc.tensor.matmul(
                        out_ps[:, ti * NP:(ti + 1) * NP],
                        wcT_sb[lo:lo + K, 0:3],
                        mask_store[lo:lo + K, p * NP:(p + 1) * NP],
                        start=True, stop=True,
                    )
                doff = bi * NP
                if p % 2 == 0:
                    nc.vector.tensor_copy(
                        out_sb2[:, doff:doff + 2 * NP], out_ps[:]
                    )
                else:
                    nc.scalar.copy(out_sb2[:, doff:doff + 2 * NP], out_ps[:])
                if bi + 2 == OBS:
                    c0b = (2 * p - OBS + 2) * NP
                    eng = [nc.sync, nc.gpsimd][(p // (OBS // 2)) % 2]
                    eng.dma_start(out_d[:, c0b:c0b + OBS * NP], out_sb2[:])

    nc.compile()
    return nc


def _get_nc(use_bias):
    if use_bias not in _CACHED:
        _CACHED[use_bias] = _build(use_bias)
    return _CACHED[use_bias]


def kernel(img, feat, coord_map, w1, b1, w2, b2, _want_trace=False):
    img = np.ascontiguousarray(np.asarray(img, dtype=np.float32))
    feat = np.ascontiguousarray(np.asarray(feat, dtype=np.float32))
    w1 = np.asarray(w1, dtype=np.float32)
    b1 = np.asarray(b1, dtype=np.float32)
    w2 = np.ascontiguousarray(np.asarray(w2, dtype=np.float32))
    b2 = np.asarray(b2, dtype=np.float32)

    use_bias = bool(np.any(b1 != 0.0))
    w1a = np.ascontiguousarray(np.concatenate([w1, b1[None, :]], axis=0))
    onesrow = np.ones((1, BS * NP), dtype=np.float32)
    b2c = np.ascontiguousarray(b2[:, None])

    nc = _get_nc(use_bias)
    in_maps = [
        {
            "feat": feat[i].reshape(CF, P),
            "img": img[i].reshape(3, P),
            "w1a": w1a,
            "w2": w2,
            "b2": b2c,
            "onesrow": onesrow,
        }
        for i in range(B)
    ]
    res = run_bass_kernel_spmd(nc, in_maps, list(range(B)), trace=_want_trace)

    transformed = np.empty((B, 3, H, W), dtype=np.float32)
    maxes = np.empty((B, K), dtype=np.float32)
    stds = np.empty((B,), dtype=np.float64)
    for i in range(B):
        r = res.results[i]
        transformed[i] = r["out"].reshape(3, H, W)
        pm = np.asarray(r["pmax"]).astype(np.float32)  # (NPAIR, 8*K) from bf16
        maxes[i] = pm.reshape(NPAIR * 8, K).max(axis=0)
        mean_k = r["sums"].reshape(K).astype(np.float64) / P
        stds[i] = np.std(mean_k, ddof=1)

    mean_max = np.float32(maxes.astype(np.float64).mean())
    std_mean = np.float32(stds.mean())
    out = (transformed, mean_max, std_mean)
    if _want_trace:
        return out, res
    return out
